# revision 23
# baseline (speedup 1.0000x reference)
"""Causal self-attention (B=1, T=4096, C=768, H=12, D=64) on 8 TRN2 NeuronCores.

Sharding: 4 head-groups x 2 query-parity sets (core c: group g=c//2 owning
heads 3g..3g+2, parity qh=c%2 owning query blocks {2j+qh}).  The host sums
the 4 head-group output partials per parity, adds b_out, and reassembles
the interleaved rows.  All SPMD cores run one identical program; per-core
variation enters only through data.

v2 (vs the fp32r baseline):
  - all PE operands are bf16 (PSUM stays fp32): FWL fast-weight-loads
    engage, matmuls run 1 cycle/row unconditionally, DMA and DVE halve.
  - x is pre-transposed AND pre-cast on the host (xT [C,T] bf16): zero
    on-device transposes.  v is produced directly in [t,d] layout by using
    xT tiles as the matmul stationary.
  - 256-query supertiles (2 parity blocks, kt span 4) cut causal
    overcompute from 22% to 9% on scores, exp and PV.
  - head-0/1 score matmuls are row-tiled (K=64 halves of the PE array via
    base partitions 0/64) so they execute concurrently.
  - two heads (or two kts) share one PSUM bank: first matmul start=True
    (whole-bank pending-zero), second start=False overwrites its
    untouched half (explicit add_dep_helper edge pins the order).  exp
    then covers [128,2,512] = 1024 free elements per ACT instruction.
  - softmax denominators still ride the ones-column in the PV stationary;
    reciprocal via reciprocal_approx_fast (~5x the DVE divide).
"""

import numpy as np
import ml_dtypes
from contextlib import ExitStack

import concourse.bass as bass  # noqa: F401
import concourse.mybir as mybir
import concourse.tile as tile
from concourse import bacc
from concourse import bass_utils
from concourse.masks import make_identity
from concourse.tile_rust import add_dep_helper

T, C, H, D = 4096, 768, 12, 64
N_CORES = 8
HPG = 3                    # heads per group
GCH = HPG * D              # 192 channels per group per tensor
TQ = T // 2                # 2048 query rows per core
NTT = T // 128             # 32 key tiles
KO = C // 128              # 6 contraction subtiles
NS = TQ // 256             # 8 query supertiles per core (256 q each)

F32 = mybir.dt.float32
F32R = mybir.dt.float32r
BF16 = mybir.dt.bfloat16
AF = mybir.ActivationFunctionType
ALU = mybir.AluOpType

_CACHE = {}
_STOP_AFTER = "full"  # "AB" | "C" | "full"
import os
_NODEPS = os.environ.get("BISECT_NODEPS", "0") == "1"
_NOSHARE = os.environ.get("BISECT_NOSHARE", "0") == "1"


def build_nc():
    nc = bacc.Bacc(
        "TRN2", target_bir_lowering=False, debug=False, num_devices=N_CORES
    )

    xT_d = nc.dram_tensor("xT", [C, T], BF16, kind="ExternalInput").ap()
    xqT_d = nc.dram_tensor("xqT", [C, TQ], BF16, kind="ExternalInput").ap()
    # packed qkv weights: per-ko concat [wk2|wk1|wv3|wq2|wq1] = 576 cols
    wp_d = nc.dram_tensor("wpack", [128, KO * 576], BF16, kind="ExternalInput").ap()
    wo_d = nc.dram_tensor("wo", [GCH, C], BF16, kind="ExternalInput").ap()
    tm_d = nc.dram_tensor("tmask", [128, 4, 256], BF16, kind="ExternalInput").ap()
    out = nc.dram_tensor("out", [C, TQ], F32, kind="ExternalOutput").ap()

    with tile.TileContext(nc) as tc, ExitStack() as ctx:
        wpool = ctx.enter_context(tc.tile_pool(name="weights", bufs=1))
        dpool = ctx.enter_context(tc.tile_pool(name="data", bufs=1))

        # --- weights / constants ---
        # weights arrive host-packed: one DMA, contiguous partition rows
        wp_sb = wpool.tile([128, KO, 576], BF16, name="wp_sb")
        nc.sync.dma_start(wp_sb[:], wp_d.rearrange("p (ko n) -> p ko n", n=576))
        W_K2, W_K1, W_V3, W_Q2, W_Q1 = (
            (0, 128), (128, 192), (192, 384), (384, 512), (512, 576)
        )
        wo_sb = [wpool.tile([64, C], BF16, name=f"wo{h}") for h in range(HPG)]
        tm_sb = wpool.tile([128, 4, 256], BF16, name="tm_sb")

        ident32 = wpool.tile([128, 128], F32, name="ident32")
        make_identity(nc, ident32[:])
        ident = wpool.tile([128, 128], BF16, name="ident")
        nc.vector.tensor_copy(ident[:], ident32[:])
        ones65_32 = wpool.tile([65, 64], F32, name="ones65_32")
        nc.vector.memset(ones65_32[:], 1.0)
        ones65 = wpool.tile([65, 64], BF16, name="ones65")
        nc.vector.tensor_copy(ones65[:], ones65_32[:])

        # --- persistent tensors ---
        qT2 = dpool.tile([128, TQ], BF16, name="qT2")     # q heads 0,1 [d,t]
        qT1 = dpool.tile([128, TQ], BF16, name="qT1")     # q head 2 (x2 halves)
        kT2 = dpool.tile([128, T], BF16, name="kT2")      # k heads 0,1
        kT1 = dpool.tile([128, T], BF16, name="kT1")      # k head 2 (x2 halves)
        vaug = dpool.tile([128, NTT, HPG, 65], BF16, name="vaug")  # [t,d]+ones
        attnT = [dpool.tile([64, TQ], BF16, name=f"aT{h}") for h in range(HPG)]
        nc.vector.memset(vaug[:, :, :, 64:65], 1.0)

        # --- phases A-D, emission interleaved per supertile so the PE
        # projects chunk s+1 while ACT exps unit s, and exp work starts as
        # soon as chunk 0 lands instead of after the whole projection pass.
        BK = 2   # kt slots per psum tile (pair units: 1 kt/bank x 2 heads;
                 # solo units: 2 kts/bank col-packed)
        LAG = 2  # batches between scores and PV
        with (
            tc.tile_pool(name="xchunk", bufs=12) as xpool,
            tc.tile_pool(name="pe", bufs=4 + LAG) as pepool,
            tc.tile_pool(name="rc", bufs=4) as rcpool,
            tc.tile_pool(name="s_ps", bufs=2, space="PSUM") as sps,
            tc.tile_pool(name="a_ps", bufs=3, space="PSUM") as apsp,
            tc.tile_pool(name="r_ps", bufs=1, space="PSUM") as rps,
            tc.tile_pool(name="ob", bufs=3) as ob_pool,
        ):
            # DMAs up front, in consumption order
            nc.sync.dma_start(tm_sb[:], tm_d[:])
            xts, xqs = [None] * 8, [None] * 4

            def dma_xt(i):
                xt = xpool.tile([128, KO, 512], BF16, tag="xt", name=f"xt{i}")
                nc.sync.dma_start(
                    xt[:],
                    xT_d[:, i * 512 : (i + 1) * 512].rearrange(
                        "(ko p) t -> p ko t", p=128
                    ),
                )
                xts[i] = xt

            def dma_xq(c):
                xq = xpool.tile([128, KO, 512], BF16, tag="xt", name=f"xq{c}")
                nc.sync.dma_start(
                    xq[:],
                    xqT_d[:, c * 512 : (c + 1) * 512].rearrange(
                        "(ko p) t -> p ko t", p=128
                    ),
                )
                xqs[c] = xq

            dma_xq(0), dma_xt(0), dma_xt(1), dma_xq(1)
            for h in range(HPG):
                nc.sync.dma_start(wo_sb[h][:], wo_d[h * 64 : (h + 1) * 64, :])
            dma_xt(2), dma_xt(3), dma_xq(2), dma_xt(4), dma_xt(5)
            dma_xq(3), dma_xt(6), dma_xt(7)

            def proj(xt, wcols, m, dest, off, dup=False):
                """dest[:, off:off+512] = wp[:, :, wcols].T @ xt over ko.

                dup: also copy the [64, 512] result into dest rows 64:128
                (head-2 operands are kept in both partition halves so the
                solo score stream can alternate PE row groups)."""
                lo, hi = wcols
                slot = sps.tile([128, BK, 512], F32, tag="s", name="projps")
                ps = slot[0:m, 0, :]
                for ko in range(KO):
                    nc.tensor.matmul(
                        ps,
                        wp_sb[:, ko, lo:hi],
                        xt[:, ko, :],
                        start=(ko == 0),
                        stop=(ko == KO - 1),
                    )
                nc.vector.tensor_copy(dest[0:m, off : off + 512], ps)
                if dup:
                    nc.vector.tensor_copy(dest[64:128, off : off + 512], ps)

            def kv_chunk(tcnk):
                xt = xts[tcnk]
                t0 = tcnk * 512
                proj(xt, W_K2, 128, kT2, t0)
                proj(xt, W_K1, 64, kT1, t0, dup=True)
                # v in [t, d] layout: xT tile stationary, Wv moving
                for tt in range(4):
                    gt = tcnk * 4 + tt
                    slot = sps.tile([128, BK, 512], F32, tag="s", name="vtps")
                    vt = slot[:, 0, 0:GCH]
                    for ko in range(KO):
                        nc.tensor.matmul(
                            vt,
                            xt[:, ko, tt * 128 : (tt + 1) * 128],
                            wp_sb[:, ko, W_V3[0] : W_V3[1]],
                            start=(ko == 0),
                            stop=(ko == KO - 1),
                        )
                    nc.vector.tensor_copy(
                        vaug[:, gt, :, 0:64],
                        vt.rearrange("p (h d) -> p h d", h=HPG),
                    )

            def q_chunk(c):
                proj(xqs[c], W_Q2, 128, qT2, c * 512)
                proj(xqs[c], W_Q1, 64, qT1, c * 512, dup=True)

            def s_lhsT(h, kt):
                ksl = slice(kt * 128, (kt + 1) * 128)
                if h == 0:
                    return kT2[0:64, ksl]
                if h == 1:
                    return kT2[64:128, ksl]
                r0 = (kt % 2) * 64
                return kT1[r0 : r0 + 64, ksl]

            def s_rhs(h, s, kt=0):
                qsl = slice(s * 256, (s + 1) * 256)
                if h == 0:
                    return qT2[0:64, qsl]
                if h == 1:
                    return qT2[64:128, qsl]
                r0 = (kt % 2) * 64
                return qT1[r0 : r0 + 64, qsl]

            def start_norm(kind, s, a_ps):
                # pair: a_ps [65, 512] (h0 cols 0:256, h1 256:512); solo 256
                w = 512 if kind == "pair" else 256
                an = rcpool.tile([65, 512], F32, tag="an")
                nc.vector.tensor_copy(an[:, 0:w], a_ps[0:65, 0:w])
                nc.vector.reciprocal(an[64:65, 0:w], an[64:65, 0:w])
                rcb = rcpool.tile([65, 512], BF16, tag="rcb")
                nc.vector.tensor_copy(rcb[64:65, 0:w], an[64:65, 0:w])
                return (kind, s, an, rcb)

            def finish_norm(kind, s, an, rcb):
                qsl = slice(s * 256, (s + 1) * 256)
                w = 512 if kind == "pair" else 256
                r_ps = rps.tile([64, 512], F32, tag="rep")
                nc.tensor.matmul(
                    r_ps[:, 0:w],
                    ones65[64:65, :],
                    rcb[64:65, 0:w],
                    start=True,
                    stop=True,
                )
                hs = (0, 1) if kind == "pair" else (2,)
                for i, h in enumerate(hs):
                    nc.vector.tensor_tensor(
                        attnT[h][:, qsl],
                        an[0:64, i * 256 : (i + 1) * 256],
                        r_ps[:, i * 256 : (i + 1) * 256],
                        ALU.mult,
                    )

            # pipeline state
            pend_pv = []    # (s, kind, nkt, a_ps, pe_t, ops)
            pend_norm = []  # (due_batch, norm_args)
            batch_no = [0]

            def flush_pv(keep):
                while len(pend_pv) > keep:
                    s, kind, nkt, a_ps, pe_t, ops, pv_first = pend_pv.pop(0)
                    for h, kt, j, c0, pc0 in ops:
                        first = (kt == 0) and not pv_first
                        m = nc.tensor.matmul(
                            a_ps[0:65, pc0 : pc0 + 256],
                            vaug[:, kt, h, 0:65],
                            pe_t[:, j, c0 : c0 + 256],
                            start=first,
                            stop=(kt == nkt - 1),
                            skip_group_check=not first,
                        )
                        if first:
                            pv_first.append(m)
                        elif kt == 0 and not _NODEPS:
                            add_dep_helper(m.ins, pv_first[0].ins, False, "aps order")
                    if any(kt == nkt - 1 for _, kt, _, _, _ in ops):
                        pend_norm.append(
                            (batch_no[0] + 4, start_norm(kind, s, a_ps))
                        )

            def flush_norms(force=False):
                while pend_norm and (force or pend_norm[0][0] <= batch_no[0]):
                    _, args = pend_norm.pop(0)
                    finish_norm(*args)

            def emit_phaseD(ts):
                tsl = slice(ts * 512, (ts + 1) * 512)
                for oc in range(C // 128):
                    ocs = slice(oc * 128, (oc + 1) * 128)
                    po = sps.tile([128, BK, 512], F32, tag="s", name="po")
                    for h in range(HPG):
                        nc.tensor.matmul(
                            po[:, 0, :],
                            wo_sb[h][:, ocs],
                            attnT[h][:, tsl],
                            start=(h == 0),
                            stop=(h == HPG - 1),
                        )
                    ob = ob_pool.tile([128, 512], F32, tag="ob")
                    nc.vector.tensor_copy(ob[:], po[:, 0, :])
                    nc.sync.dma_start(out[ocs, tsl], ob[:])

            unit_list = []
            for s in range(NS):
                unit_list.append(("kv", s))
                if s == 0:
                    unit_list.append(("q", 0))
                if _STOP_AFTER != "AB":
                    unit_list.append((s, "pair"))
                    unit_list.append((s, "solo"))
                if s % 2 == 1 and s < 7:
                    unit_list.append(("q", (s + 1) // 2))

            for s, kind in unit_list:
                if s == "kv":
                    kv_chunk(kind)
                    continue
                if s == "q":
                    q_chunk(kind)
                    continue
                nkt = 4 * s + 4
                flush_norms(force=True)
                # phase D for query chunk ts slots in once the norms for
                # its supertiles (2ts, 2ts+1) are guaranteed flushed: at the
                # SOLO unit of supertile 2ts+2 (the pair unit of 2ts+2 pops
                # all of 2ts+1's pending PV batches through the LAG window,
                # and this unit's force-flush above emitted the norms).
                if (
                    kind == "solo" and s >= 2 and s % 2 == 0
                    and _STOP_AFTER == "full"
                ):
                    emit_phaseD((s - 2) // 2)
                a_ps = apsp.tile([65, 512], F32, tag="attn", name="a_ps")
                pv_first = []
                step = 2 if kind == "pair" else 4
                for kt0 in range(0, nkt, step):
                    kts = list(range(kt0, kt0 + step))
                    bs = sps.tile([128, BK, 512], F32, tag="s")
                    # ops: (head, kt, bank j, score col c0, pv col pc0)
                    if kind == "pair":
                        # bank h holds head h's kt pair col-packed; same-bank
                        # writers share tile_position, row-tiled heads write
                        # different banks
                        ops = []
                        for i, kt in enumerate(kts):
                            ops += [(0, kt, 0, i * 256, 0), (1, kt, 1, i * 256, 256)]
                    else:
                        # bank j holds same-parity kts (same PE row group per
                        # bank); adjacent ops alternate row groups -> overlap
                        ops = [
                            (2, kt0 + 2 * i + j, j, i * 256, 0)
                            for i in range(2)
                            for j in range(2)
                        ]
                    bank_first = {}
                    for h, kt, j, c0, pc0 in ops:
                        tail = kt >= 4 * s
                        first = j not in bank_first
                        m = nc.tensor.matmul(
                            bs[:, j, c0 : c0 + 256],
                            s_lhsT(h, kt), s_rhs(h, s, kt),
                            start=first, stop=not tail,
                            skip_group_check=not first,
                        )
                        if first:
                            bank_first[j] = m
                        elif not _NODEPS:
                            add_dep_helper(m.ins, bank_first[j].ins, False, "bank order")
                        if tail:
                            r = kt - 4 * s
                            nc.tensor.matmul(
                                bs[:, j, c0 : c0 + 256], ident[:], tm_sb[:, r, :],
                                start=False, stop=True,
                                skip_group_check=True,
                            )
                    batch_no[0] += 1
                    flush_pv(LAG)
                    flush_norms()
                    pe_t = pepool.tile([128, BK, 512], BF16, tag="pe")
                    nc.scalar.activation(pe_t[:], bs[:], AF.Exp, scale=0.125)
                    pend_pv.append((s, kind, nkt, a_ps, pe_t, ops, pv_first))
            flush_pv(0)
            flush_norms(force=True)
            if _STOP_AFTER == "full":
                emit_phaseD(3)

    nc.compile()
    return nc


def _get_nc():
    if "nc" not in _CACHE:
        _CACHE["nc"] = build_nc()
    return _CACHE["nc"]


BF = ml_dtypes.bfloat16


def pack_w(w):
    """[C, n] -> [128, KO*n] so each SBUF partition row is contiguous."""
    n = w.shape[1]
    return np.ascontiguousarray(
        w.reshape(KO, 128, n).transpose(1, 0, 2).reshape(128, KO * n)
    ).astype(BF)


def make_in_maps(inputs):
    """Shard full inputs into 8 per-core input maps.

    xT [C, T] is the host-transposed bf16 x, shared by all cores (k/v need
    every key row).  xqT [C, TQ] is the parity-gathered query view: core
    parity qh owns global 128-row q blocks {2j+qh}, laid out ascending.

    tmask [128k, r, 256q] covers the 4 tail kts (r = kt - 4s) of each
    256-query supertile s.  Local q block j (j=0,1) of supertile s is
    global block 4s+2j+qh; tail kt 4s+r is global key block 4s+r, so
    delta = r - 2j - qh: 0 -> diagonal triangle mask, >0 -> fully masked,
    <0 -> keep (zeros).
    """
    x = np.ascontiguousarray(np.asarray(inputs["x"], dtype=np.float32)).reshape(T, C)
    W_qkv = np.asarray(inputs["W_qkv"], dtype=np.float32)
    W_out = np.asarray(inputs["W_out"], dtype=np.float32)

    NEG = np.float32(-1e9)
    diag_add = np.where(
        np.arange(128)[None, :] >= np.arange(128)[:, None], np.float32(0), NEG
    )  # [k, q]: keep q >= k

    xT = np.ascontiguousarray(x.T).astype(BF)  # [C, T]
    xr = x.reshape(NTT, 128, C)
    xqT = {
        qh: np.ascontiguousarray(xr[qh::2].reshape(TQ, C).T).astype(BF)
        for qh in (0, 1)
    }

    tmask = {}
    for qh in (0, 1):
        m = np.zeros((128, 4, 256), np.float32)
        for r in range(4):
            for j in range(2):
                delta = r - 2 * j - qh
                blk = m[:, r, j * 128 : (j + 1) * 128]
                if delta == 0:
                    blk[:] = diag_add
                elif delta > 0:
                    blk[:] = NEG
        tmask[qh] = m.astype(BF)

    in_maps = []
    for c in range(N_CORES):
        g, qh = c // 2, c % 2
        in_maps.append(
            {
                "xT": xT,
                "xqT": xqT[qh],
                "wpack": pack_w(
                    np.concatenate(
                        [
                            W_qkv[:, 1 * C + g * GCH : 1 * C + (g + 1) * GCH],
                            W_qkv[:, 2 * C + g * GCH : 2 * C + (g + 1) * GCH],
                            W_qkv[:, 0 * C + g * GCH : 0 * C + (g + 1) * GCH],
                        ],
                        axis=1,
                    )
                ),
                "wo": np.ascontiguousarray(W_out[g * GCH : (g + 1) * GCH, :]).astype(BF),
                "tmask": tmask[qh],
            }
        )
    return in_maps


def combine_outputs(parts, b_out):
    """Sum head-group partials per parity, reassemble rows, add bias."""
    NQT = TQ // 128
    out = np.zeros((T, C), np.float32)
    orow = out.reshape(NTT, 128, C)
    for qh in (0, 1):
        acc = parts[qh].astype(np.float32).copy()
        for g in range(1, 4):
            acc += parts[2 * g + qh]
        orow[qh::2] = np.ascontiguousarray(acc.T).reshape(NQT, 128, C)
    out += np.asarray(b_out, dtype=np.float32)[None, :]
    return out.reshape(1, T, C)


def _run(inputs, trace=False, tmpdir=None):
    nc = _get_nc()
    in_maps = make_in_maps(inputs)
    res = bass_utils.run_bass_kernel_spmd(
        nc, in_maps, core_ids=list(range(N_CORES)), trace=trace, tmpdir=tmpdir
    )
    parts = [np.asarray(res.results[c]["out"]) for c in range(N_CORES)]
    return combine_outputs(parts, inputs["b_out"]), res


def kernel(**inputs):
    out, _ = _run(inputs)
    return out


# revision 24
# speedup vs baseline: 1.0519x; 1.0519x over previous
"""Causal self-attention (B=1, T=4096, C=768, H=12, D=64) on 8 TRN2 NeuronCores.

Sharding: 4 head-groups x 2 query-parity sets (core c: group g=c//2 owning
heads 3g..3g+2, parity qh=c%2 owning query blocks {2j+qh}).  The host sums
the 4 head-group output partials per parity, adds b_out, and reassembles
the interleaved rows.  All SPMD cores run one identical program; per-core
variation enters only through data.

v2 (vs the fp32r baseline):
  - all PE operands are bf16 (PSUM stays fp32): FWL fast-weight-loads
    engage, matmuls run 1 cycle/row unconditionally, DMA and DVE halve.
  - x is pre-transposed AND pre-cast on the host (xT [C,T] bf16): zero
    on-device transposes.  v is produced directly in [t,d] layout by using
    xT tiles as the matmul stationary.
  - 256-query supertiles (2 parity blocks, kt span 4) cut causal
    overcompute from 22% to 9% on scores, exp and PV.
  - head-0/1 score matmuls are row-tiled (K=64 halves of the PE array via
    base partitions 0/64) so they execute concurrently.
  - two heads (or two kts) share one PSUM bank: first matmul start=True
    (whole-bank pending-zero), second start=False overwrites its
    untouched half (explicit add_dep_helper edge pins the order).  exp
    then covers [128,2,512] = 1024 free elements per ACT instruction.
  - softmax denominators still ride the ones-column in the PV stationary;
    reciprocal via reciprocal_approx_fast (~5x the DVE divide).
"""

import numpy as np
import ml_dtypes
from contextlib import ExitStack

import concourse.bass as bass  # noqa: F401
import concourse.mybir as mybir
import concourse.tile as tile
from concourse import bacc
from concourse import bass_utils
from concourse.masks import make_identity
from concourse.tile_rust import add_dep_helper

T, C, H, D = 4096, 768, 12, 64
N_CORES = 8
HPG = 3                    # heads per group
GCH = HPG * D              # 192 channels per group per tensor
TQ = T // 2                # 2048 query rows per core
NTT = T // 128             # 32 key tiles
KO = C // 128              # 6 contraction subtiles
NS = TQ // 256             # 8 query supertiles per core (256 q each)

F32 = mybir.dt.float32
F32R = mybir.dt.float32r
BF16 = mybir.dt.bfloat16
AF = mybir.ActivationFunctionType
ALU = mybir.AluOpType

_CACHE = {}
_STOP_AFTER = "full"  # "AB" | "C" | "full"
import os
_NODEPS = os.environ.get("BISECT_NODEPS", "0") == "1"
_NOSHARE = os.environ.get("BISECT_NOSHARE", "0") == "1"


def build_nc():
    nc = bacc.Bacc(
        "TRN2", target_bir_lowering=False, debug=False, num_devices=N_CORES
    )

    xT_d = nc.dram_tensor("xT", [C, T], BF16, kind="ExternalInput").ap()
    xqT_d = nc.dram_tensor("xqT", [C, TQ], BF16, kind="ExternalInput").ap()
    # packed qkv weights: per-ko concat [wk2|wk1|wv3|wq2|wq1] = 576 cols
    wp_d = nc.dram_tensor("wpack", [128, KO * 576], BF16, kind="ExternalInput").ap()
    wo_d = nc.dram_tensor("wo", [GCH, C], BF16, kind="ExternalInput").ap()
    tm_d = nc.dram_tensor("tmask", [128, 4, 256], BF16, kind="ExternalInput").ap()
    out = nc.dram_tensor("out", [C, TQ], F32, kind="ExternalOutput").ap()

    with tile.TileContext(nc) as tc, ExitStack() as ctx:
        wpool = ctx.enter_context(tc.tile_pool(name="weights", bufs=1))
        dpool = ctx.enter_context(tc.tile_pool(name="data", bufs=1))

        # --- weights / constants ---
        # weights arrive host-packed: one DMA, contiguous partition rows
        wp_sb = wpool.tile([128, KO, 576], BF16, name="wp_sb")
        nc.sync.dma_start(wp_sb[:], wp_d.rearrange("p (ko n) -> p ko n", n=576))
        W_K2, W_K1, W_V3, W_Q2, W_Q1 = (
            (0, 128), (128, 192), (192, 384), (384, 512), (512, 576)
        )
        wo_sb = [wpool.tile([64, C], BF16, name=f"wo{h}") for h in range(HPG)]
        tm_sb = wpool.tile([128, 4, 256], BF16, name="tm_sb")

        ident32 = wpool.tile([128, 128], F32, name="ident32")
        make_identity(nc, ident32[:])
        ident = wpool.tile([128, 128], BF16, name="ident")
        nc.vector.tensor_copy(ident[:], ident32[:])
        ones65_32 = wpool.tile([65, 64], F32, name="ones65_32")
        nc.vector.memset(ones65_32[:], 1.0)
        ones65 = wpool.tile([65, 64], BF16, name="ones65")
        nc.vector.tensor_copy(ones65[:], ones65_32[:])

        # --- persistent tensors ---
        qT2 = dpool.tile([128, TQ], BF16, name="qT2")     # q heads 0,1 [d,t]
        qT1 = dpool.tile([64, TQ], BF16, name="qT1")      # q head 2
        kT2 = dpool.tile([128, T], BF16, name="kT2")      # k heads 0,1
        kT1 = dpool.tile([64, T], BF16, name="kT1")       # k head 2
        vaug = dpool.tile([128, NTT, HPG, 65], BF16, name="vaug")  # [t,d]+ones
        attnT = [dpool.tile([64, TQ], BF16, name=f"aT{h}") for h in range(HPG)]
        nc.vector.memset(vaug[:, :, :, 64:65], 1.0)

        # --- phases A-D, emission interleaved per supertile so the PE
        # projects chunk s+1 while ACT exps unit s, and exp work starts as
        # soon as chunk 0 lands instead of after the whole projection pass.
        BK = 2   # kt slots per psum tile (pair units: 1 kt/bank x 2 heads;
                 # solo units: 2 kts/bank col-packed)
        LAG = 2  # batches between scores and PV
        with (
            tc.tile_pool(name="xchunk", bufs=12) as xpool,
            tc.tile_pool(name="pe", bufs=4 + LAG) as pepool,
            tc.tile_pool(name="rc", bufs=4) as rcpool,
            tc.tile_pool(name="s_ps", bufs=2, space="PSUM") as sps,
            tc.tile_pool(name="a_ps", bufs=3, space="PSUM") as apsp,
            tc.tile_pool(name="r_ps", bufs=1, space="PSUM") as rps,
            tc.tile_pool(name="ob", bufs=3) as ob_pool,
        ):
            # DMAs up front, in consumption order
            nc.sync.dma_start(tm_sb[:], tm_d[:])
            xts, xqs = [None] * 8, [None] * 4

            def dma_xt(i):
                xt = xpool.tile([128, KO, 512], BF16, tag="xt", name=f"xt{i}")
                nc.sync.dma_start(
                    xt[:],
                    xT_d[:, i * 512 : (i + 1) * 512].rearrange(
                        "(ko p) t -> p ko t", p=128
                    ),
                )
                xts[i] = xt

            def dma_xq(c):
                xq = xpool.tile([128, KO, 512], BF16, tag="xt", name=f"xq{c}")
                nc.sync.dma_start(
                    xq[:],
                    xqT_d[:, c * 512 : (c + 1) * 512].rearrange(
                        "(ko p) t -> p ko t", p=128
                    ),
                )
                xqs[c] = xq

            dma_xq(0), dma_xt(0), dma_xt(1), dma_xq(1)
            for h in range(HPG):
                nc.sync.dma_start(wo_sb[h][:], wo_d[h * 64 : (h + 1) * 64, :])
            dma_xt(2), dma_xt(3), dma_xq(2), dma_xt(4), dma_xt(5)
            dma_xq(3), dma_xt(6), dma_xt(7)

            def proj(xt, wcols, m, dest, off):
                """dest[:, off:off+512] = wp[:, :, wcols].T @ xt over ko."""
                lo, hi = wcols
                slot = sps.tile([128, BK, 512], F32, tag="s", name="projps")
                ps = slot[0:m, 0, :]
                for ko in range(KO):
                    nc.tensor.matmul(
                        ps,
                        wp_sb[:, ko, lo:hi],
                        xt[:, ko, :],
                        start=(ko == 0),
                        stop=(ko == KO - 1),
                    )
                nc.vector.tensor_copy(dest[:, off : off + 512], ps)

            def kv_chunk(tcnk):
                xt = xts[tcnk]
                t0 = tcnk * 512
                proj(xt, W_K2, 128, kT2, t0)
                proj(xt, W_K1, 64, kT1, t0)
                # v in [t, d] layout: xT tile stationary, Wv moving
                for tt in range(4):
                    gt = tcnk * 4 + tt
                    slot = sps.tile([128, BK, 512], F32, tag="s", name="vtps")
                    vt = slot[:, 0, 0:GCH]
                    for ko in range(KO):
                        nc.tensor.matmul(
                            vt,
                            xt[:, ko, tt * 128 : (tt + 1) * 128],
                            wp_sb[:, ko, W_V3[0] : W_V3[1]],
                            start=(ko == 0),
                            stop=(ko == KO - 1),
                        )
                    nc.vector.tensor_copy(
                        vaug[:, gt, :, 0:64],
                        vt.rearrange("p (h d) -> p h d", h=HPG),
                    )

            def q_chunk(c):
                proj(xqs[c], W_Q2, 128, qT2, c * 512)
                proj(xqs[c], W_Q1, 64, qT1, c * 512)

            def s_lhsT(h, kt):
                ksl = slice(kt * 128, (kt + 1) * 128)
                if h == 0:
                    return kT2[0:64, ksl]
                if h == 1:
                    return kT2[64:128, ksl]
                return kT1[0:64, ksl]

            def s_rhs(h, s, kt=0):
                qsl = slice(s * 256, (s + 1) * 256)
                if h == 0:
                    return qT2[0:64, qsl]
                if h == 1:
                    return qT2[64:128, qsl]
                return qT1[0:64, qsl]

            def start_norm(kind, s, a_ps):
                # pair: a_ps [65, 512] (h0 cols 0:256, h1 256:512); solo 256
                w = 512 if kind == "pair" else 256
                an = rcpool.tile([65, 512], F32, tag="an")
                nc.vector.tensor_copy(an[:, 0:w], a_ps[0:65, 0:w])
                nc.vector.reciprocal(an[64:65, 0:w], an[64:65, 0:w])
                rcb = rcpool.tile([65, 512], BF16, tag="rcb")
                nc.vector.tensor_copy(rcb[64:65, 0:w], an[64:65, 0:w])
                return (kind, s, an, rcb)

            def finish_norm(kind, s, an, rcb):
                qsl = slice(s * 256, (s + 1) * 256)
                w = 512 if kind == "pair" else 256
                r_ps = rps.tile([64, 512], F32, tag="rep")
                nc.tensor.matmul(
                    r_ps[:, 0:w],
                    ones65[64:65, :],
                    rcb[64:65, 0:w],
                    start=True,
                    stop=True,
                )
                hs = (0, 1) if kind == "pair" else (2,)
                for i, h in enumerate(hs):
                    nc.vector.tensor_tensor(
                        attnT[h][:, qsl],
                        an[0:64, i * 256 : (i + 1) * 256],
                        r_ps[:, i * 256 : (i + 1) * 256],
                        ALU.mult,
                    )

            # pipeline state
            pend_pv = []    # (s, kind, nkt, a_ps, pe_t, ops)
            pend_norm = []  # (due_batch, norm_args)
            batch_no = [0]

            def flush_pv(keep):
                while len(pend_pv) > keep:
                    s, kind, nkt, a_ps, pe_t, ops, pv_first = pend_pv.pop(0)
                    for h, kt, j, c0, pc0 in ops:
                        first = (kt == 0) and not pv_first
                        m = nc.tensor.matmul(
                            a_ps[0:65, pc0 : pc0 + 256],
                            vaug[:, kt, h, 0:65],
                            pe_t[:, j, c0 : c0 + 256],
                            start=first,
                            stop=(kt == nkt - 1),
                            skip_group_check=not first,
                        )
                        if first:
                            pv_first.append(m)
                        elif kt == 0 and not _NODEPS:
                            add_dep_helper(m.ins, pv_first[0].ins, False, "aps order")
                    if any(kt == nkt - 1 for _, kt, _, _, _ in ops):
                        pend_norm.append(
                            (batch_no[0] + 4, start_norm(kind, s, a_ps))
                        )

            def flush_norms(force=False):
                while pend_norm and (force or pend_norm[0][0] <= batch_no[0]):
                    _, args = pend_norm.pop(0)
                    finish_norm(*args)

            def emit_phaseD(ts):
                tsl = slice(ts * 512, (ts + 1) * 512)
                for oc in range(C // 128):
                    ocs = slice(oc * 128, (oc + 1) * 128)
                    po = sps.tile([128, BK, 512], F32, tag="s", name="po")
                    for h in range(HPG):
                        nc.tensor.matmul(
                            po[:, 0, :],
                            wo_sb[h][:, ocs],
                            attnT[h][:, tsl],
                            start=(h == 0),
                            stop=(h == HPG - 1),
                        )
                    ob = ob_pool.tile([128, 512], F32, tag="ob")
                    nc.vector.tensor_copy(ob[:], po[:, 0, :])
                    nc.sync.dma_start(out[ocs, tsl], ob[:])

            unit_list = []
            for s in range(NS):
                unit_list.append(("kv", s))
                if s == 0:
                    unit_list.append(("q", 0))
                if _STOP_AFTER != "AB":
                    unit_list.append((s, "pair"))
                    unit_list.append((s, "solo"))
                if s % 2 == 1 and s < 7:
                    unit_list.append(("q", (s + 1) // 2))

            for s, kind in unit_list:
                if s == "kv":
                    kv_chunk(kind)
                    continue
                if s == "q":
                    q_chunk(kind)
                    continue
                nkt = 4 * s + 4
                flush_norms(force=True)
                # phase D for query chunk ts slots in once the norms for
                # its supertiles (2ts, 2ts+1) are guaranteed flushed: at the
                # SOLO unit of supertile 2ts+2 (the pair unit of 2ts+2 pops
                # all of 2ts+1's pending PV batches through the LAG window,
                # and this unit's force-flush above emitted the norms).
                if (
                    kind == "solo" and s >= 2 and s % 2 == 0
                    and _STOP_AFTER == "full"
                ):
                    emit_phaseD((s - 2) // 2)
                a_ps = apsp.tile([65, 512], F32, tag="attn", name="a_ps")
                pv_first = []
                step = 2 if kind == "pair" else 4
                for kt0 in range(0, nkt, step):
                    kts = list(range(kt0, kt0 + step))
                    bs = sps.tile([128, BK, 512], F32, tag="s")
                    # ops: (head, kt, bank j, score col c0, pv col pc0)
                    if kind == "pair":
                        # bank h holds head h's kt pair col-packed; same-bank
                        # writers share tile_position, row-tiled heads write
                        # different banks
                        ops = []
                        for i, kt in enumerate(kts):
                            ops += [(0, kt, 0, i * 256, 0), (1, kt, 1, i * 256, 256)]
                    else:
                        ops = [
                            (2, kt, idx // 2, (idx % 2) * 256, 0)
                            for idx, kt in enumerate(kts)
                        ]
                    bank_first = {}
                    for h, kt, j, c0, pc0 in ops:
                        tail = kt >= 4 * s
                        first = j not in bank_first
                        m = nc.tensor.matmul(
                            bs[:, j, c0 : c0 + 256],
                            s_lhsT(h, kt), s_rhs(h, s, kt),
                            start=first, stop=not tail,
                            skip_group_check=not first,
                        )
                        if first:
                            bank_first[j] = m
                        elif not _NODEPS:
                            add_dep_helper(m.ins, bank_first[j].ins, False, "bank order")
                        if tail:
                            r = kt - 4 * s
                            nc.tensor.matmul(
                                bs[:, j, c0 : c0 + 256], ident[:], tm_sb[:, r, :],
                                start=False, stop=True,
                                skip_group_check=True,
                            )
                    batch_no[0] += 1
                    flush_pv(LAG)
                    flush_norms()
                    pe_t = pepool.tile([128, BK, 512], BF16, tag="pe")
                    nc.scalar.activation(pe_t[:], bs[:], AF.Exp, scale=0.125)
                    pend_pv.append((s, kind, nkt, a_ps, pe_t, ops, pv_first))
            flush_pv(0)
            flush_norms(force=True)
            if _STOP_AFTER == "full":
                emit_phaseD(3)

    nc.compile()
    return nc


def _get_nc():
    if "nc" not in _CACHE:
        _CACHE["nc"] = build_nc()
    return _CACHE["nc"]


BF = ml_dtypes.bfloat16


def pack_w(w):
    """[C, n] -> [128, KO*n] so each SBUF partition row is contiguous."""
    n = w.shape[1]
    return np.ascontiguousarray(
        w.reshape(KO, 128, n).transpose(1, 0, 2).reshape(128, KO * n)
    ).astype(BF)


def make_in_maps(inputs):
    """Shard full inputs into 8 per-core input maps.

    xT [C, T] is the host-transposed bf16 x, shared by all cores (k/v need
    every key row).  xqT [C, TQ] is the parity-gathered query view: core
    parity qh owns global 128-row q blocks {2j+qh}, laid out ascending.

    tmask [128k, r, 256q] covers the 4 tail kts (r = kt - 4s) of each
    256-query supertile s.  Local q block j (j=0,1) of supertile s is
    global block 4s+2j+qh; tail kt 4s+r is global key block 4s+r, so
    delta = r - 2j - qh: 0 -> diagonal triangle mask, >0 -> fully masked,
    <0 -> keep (zeros).
    """
    x = np.ascontiguousarray(np.asarray(inputs["x"], dtype=np.float32)).reshape(T, C)
    W_qkv = np.asarray(inputs["W_qkv"], dtype=np.float32)
    W_out = np.asarray(inputs["W_out"], dtype=np.float32)

    NEG = np.float32(-1e9)
    diag_add = np.where(
        np.arange(128)[None, :] >= np.arange(128)[:, None], np.float32(0), NEG
    )  # [k, q]: keep q >= k

    xT = np.ascontiguousarray(x.T).astype(BF)  # [C, T]
    xr = x.reshape(NTT, 128, C)
    xqT = {
        qh: np.ascontiguousarray(xr[qh::2].reshape(TQ, C).T).astype(BF)
        for qh in (0, 1)
    }

    tmask = {}
    for qh in (0, 1):
        m = np.zeros((128, 4, 256), np.float32)
        for r in range(4):
            for j in range(2):
                delta = r - 2 * j - qh
                blk = m[:, r, j * 128 : (j + 1) * 128]
                if delta == 0:
                    blk[:] = diag_add
                elif delta > 0:
                    blk[:] = NEG
        tmask[qh] = m.astype(BF)

    in_maps = []
    for c in range(N_CORES):
        g, qh = c // 2, c % 2
        in_maps.append(
            {
                "xT": xT,
                "xqT": xqT[qh],
                "wpack": pack_w(
                    np.concatenate(
                        [
                            W_qkv[:, 1 * C + g * GCH : 1 * C + (g + 1) * GCH],
                            W_qkv[:, 2 * C + g * GCH : 2 * C + (g + 1) * GCH],
                            W_qkv[:, 0 * C + g * GCH : 0 * C + (g + 1) * GCH],
                        ],
                        axis=1,
                    )
                ),
                "wo": np.ascontiguousarray(W_out[g * GCH : (g + 1) * GCH, :]).astype(BF),
                "tmask": tmask[qh],
            }
        )
    return in_maps


def combine_outputs(parts, b_out):
    """Sum head-group partials per parity, reassemble rows, add bias."""
    NQT = TQ // 128
    out = np.zeros((T, C), np.float32)
    orow = out.reshape(NTT, 128, C)
    for qh in (0, 1):
        acc = parts[qh].astype(np.float32).copy()
        for g in range(1, 4):
            acc += parts[2 * g + qh]
        orow[qh::2] = np.ascontiguousarray(acc.T).reshape(NQT, 128, C)
    out += np.asarray(b_out, dtype=np.float32)[None, :]
    return out.reshape(1, T, C)


def _run(inputs, trace=False, tmpdir=None):
    nc = _get_nc()
    in_maps = make_in_maps(inputs)
    res = bass_utils.run_bass_kernel_spmd(
        nc, in_maps, core_ids=list(range(N_CORES)), trace=trace, tmpdir=tmpdir
    )
    parts = [np.asarray(res.results[c]["out"]) for c in range(N_CORES)]
    return combine_outputs(parts, inputs["b_out"]), res


def kernel(**inputs):
    out, _ = _run(inputs)
    return out


# revision 25
# speedup vs baseline: 1.1408x; 1.0845x over previous
"""Causal self-attention (B=1, T=4096, C=768, H=12, D=64) on 8 TRN2 NeuronCores.

Sharding: 4 head-groups x 2 query-parity sets (core c: group g=c//2 owning
heads 3g..3g+2, parity qh=c%2 owning query blocks {2j+qh}).  The host sums
the 4 head-group output partials per parity, adds b_out, and reassembles
the interleaved rows.  All SPMD cores run one identical program; per-core
variation enters only through data.

v2 (vs the fp32r baseline):
  - all PE operands are bf16 (PSUM stays fp32): FWL fast-weight-loads
    engage, matmuls run 1 cycle/row unconditionally, DMA and DVE halve.
  - x is pre-transposed AND pre-cast on the host (xT [C,T] bf16): zero
    on-device transposes.  v is produced directly in [t,d] layout by using
    xT tiles as the matmul stationary.
  - 256-query supertiles (2 parity blocks, kt span 4) cut causal
    overcompute from 22% to 9% on scores, exp and PV.
  - head-0/1 score matmuls are row-tiled (K=64 halves of the PE array via
    base partitions 0/64) so they execute concurrently.
  - two heads (or two kts) share one PSUM bank: first matmul start=True
    (whole-bank pending-zero), second start=False overwrites its
    untouched half (explicit add_dep_helper edge pins the order).  exp
    then covers [128,2,512] = 1024 free elements per ACT instruction.
  - softmax denominators still ride the ones-column in the PV stationary;
    reciprocal via reciprocal_approx_fast (~5x the DVE divide).
"""

import numpy as np
import ml_dtypes
from contextlib import ExitStack

import concourse.bass as bass  # noqa: F401
import concourse.mybir as mybir
import concourse.tile as tile
from concourse import bacc
from concourse import bass_utils
from concourse.masks import make_identity
from concourse.tile_rust import add_dep_helper

T, C, H, D = 4096, 768, 12, 64
N_CORES = 8
HPG = 3                    # heads per group
GCH = HPG * D              # 192 channels per group per tensor
TQ = T // 2                # 2048 query rows per core
NTT = T // 128             # 32 key tiles
KO = C // 128              # 6 contraction subtiles
NS = TQ // 256             # 8 query supertiles per core (256 q each)

F32 = mybir.dt.float32
F32R = mybir.dt.float32r
BF16 = mybir.dt.bfloat16
AF = mybir.ActivationFunctionType
ALU = mybir.AluOpType

_CACHE = {}
_STOP_AFTER = "full"  # "AB" | "C" | "full"
import os
_NODEPS = os.environ.get("BISECT_NODEPS", "0") == "1"
_NOSHARE = os.environ.get("BISECT_NOSHARE", "0") == "1"


def build_nc():
    nc = bacc.Bacc(
        "TRN2", target_bir_lowering=False, debug=False, num_devices=N_CORES
    )

    xT_d = nc.dram_tensor("xT", [C, T], BF16, kind="ExternalInput").ap()
    xqT_d = nc.dram_tensor("xqT", [C, TQ], BF16, kind="ExternalInput").ap()
    # packed qkv weights: per-ko concat [wk2|wk1|wv3|wq2|wq1] = 576 cols
    wp_d = nc.dram_tensor("wpack", [128, KO * 576], BF16, kind="ExternalInput").ap()
    wo_d = nc.dram_tensor("wo", [GCH, C], BF16, kind="ExternalInput").ap()
    tm_d = nc.dram_tensor("tmask", [128, 4, 256], BF16, kind="ExternalInput").ap()
    out = nc.dram_tensor("out", [C, TQ], F32, kind="ExternalOutput").ap()

    with tile.TileContext(nc) as tc, ExitStack() as ctx:
        wpool = ctx.enter_context(tc.tile_pool(name="weights", bufs=1))
        dpool = ctx.enter_context(tc.tile_pool(name="data", bufs=1))

        # --- weights / constants ---
        # weights arrive host-packed: one DMA, contiguous partition rows
        wp_sb = wpool.tile([128, KO, 576], BF16, name="wp_sb")
        nc.sync.dma_start(wp_sb[:], wp_d.rearrange("p (ko n) -> p ko n", n=576))
        W_K2, W_K1, W_V3, W_Q2, W_Q1 = (
            (0, 128), (128, 192), (192, 384), (384, 512), (512, 576)
        )
        wo_sb = [wpool.tile([64, C], BF16, name=f"wo{h}") for h in range(HPG)]
        tm_sb = wpool.tile([128, 4, 256], BF16, name="tm_sb")

        ident32 = wpool.tile([128, 128], F32, name="ident32")
        make_identity(nc, ident32[:])
        ident = wpool.tile([128, 128], BF16, name="ident")
        nc.vector.tensor_copy(ident[:], ident32[:])
        ones65_32 = wpool.tile([65, 64], F32, name="ones65_32")
        nc.vector.memset(ones65_32[:], 1.0)
        ones65 = wpool.tile([65, 64], BF16, name="ones65")
        nc.vector.tensor_copy(ones65[:], ones65_32[:])

        # --- persistent tensors ---
        qT2 = dpool.tile([128, TQ], BF16, name="qT2")     # q heads 0,1 [d,t]
        qT1 = dpool.tile([64, TQ], BF16, name="qT1")      # q head 2
        kT2 = dpool.tile([128, T], BF16, name="kT2")      # k heads 0,1
        kT1 = dpool.tile([64, T], BF16, name="kT1")       # k head 2
        vaug = dpool.tile([128, NTT, HPG, 65], BF16, name="vaug")  # [t,d]+ones
        attnT = [dpool.tile([64, TQ], BF16, name=f"aT{h}") for h in range(HPG)]
        nc.vector.memset(vaug[:, :, :, 64:65], 1.0)

        # --- phases A-D, emission interleaved per supertile so the PE
        # projects chunk s+1 while ACT exps unit s, and exp work starts as
        # soon as chunk 0 lands instead of after the whole projection pass.
        BK = 2   # kt slots per psum tile (pair units: 1 kt/bank x 2 heads;
                 # solo units: 2 kts/bank col-packed)
        LAG = 2  # batches between scores and PV
        with (
            tc.tile_pool(name="xchunk", bufs=12) as xpool,
            tc.tile_pool(name="pe", bufs=4 + LAG) as pepool,
            tc.tile_pool(name="rc", bufs=4) as rcpool,
            tc.tile_pool(name="s_ps", bufs=2, space="PSUM") as sps,
            tc.tile_pool(name="a_ps", bufs=2, space="PSUM") as apsp,
            tc.tile_pool(name="x_ps", bufs=2, space="PSUM") as aux,
            tc.tile_pool(name="ob", bufs=3) as ob_pool,
        ):
            # DMAs up front, in consumption order
            nc.sync.dma_start(tm_sb[:], tm_d[:])
            xts, xqs = [None] * 8, [None] * 4

            def dma_xt(i):
                xt = xpool.tile([128, KO, 512], BF16, tag="xt", name=f"xt{i}")
                nc.sync.dma_start(
                    xt[:],
                    xT_d[:, i * 512 : (i + 1) * 512].rearrange(
                        "(ko p) t -> p ko t", p=128
                    ),
                )
                xts[i] = xt

            def dma_xq(c):
                xq = xpool.tile([128, KO, 512], BF16, tag="xt", name=f"xq{c}")
                nc.sync.dma_start(
                    xq[:],
                    xqT_d[:, c * 512 : (c + 1) * 512].rearrange(
                        "(ko p) t -> p ko t", p=128
                    ),
                )
                xqs[c] = xq

            dma_xq(0), dma_xt(0), dma_xt(1), dma_xq(1)
            for h in range(HPG):
                nc.sync.dma_start(wo_sb[h][:], wo_d[h * 64 : (h + 1) * 64, :])
            dma_xt(2), dma_xt(3), dma_xq(2), dma_xt(4), dma_xt(5)
            dma_xq(3), dma_xt(6), dma_xt(7)

            def proj(xt, wcols, m, dest, off):
                """dest[:, off:off+512] = wp[:, :, wcols].T @ xt over ko."""
                lo, hi = wcols
                slot = aux.tile([128, 512], F32, tag="aux", name="projps")
                ps = slot[0:m, :]
                for ko in range(KO):
                    nc.tensor.matmul(
                        ps,
                        wp_sb[:, ko, lo:hi],
                        xt[:, ko, :],
                        start=(ko == 0),
                        stop=(ko == KO - 1),
                    )
                nc.vector.tensor_copy(dest[:, off : off + 512], ps)

            def kv_chunk(tcnk):
                xt = xts[tcnk]
                t0 = tcnk * 512
                proj(xt, W_K2, 128, kT2, t0)
                proj(xt, W_K1, 64, kT1, t0)
                # v in [t, d] layout: xT tile stationary, Wv moving;
                # two t-tiles col-packed per psum bank (same tile_position)
                for tp2 in range(2):
                    slot = aux.tile([128, 512], F32, tag="aux", name="vtps")
                    anchor = None
                    for i in range(2):
                        tt = tp2 * 2 + i
                        vt = slot[:, i * 256 : i * 256 + GCH]
                        for ko in range(KO):
                            m = nc.tensor.matmul(
                                vt,
                                xt[:, ko, tt * 128 : (tt + 1) * 128],
                                wp_sb[:, ko, W_V3[0] : W_V3[1]],
                                start=(ko == 0 and i == 0),
                                stop=(ko == KO - 1),
                                skip_group_check=(i == 1),
                            )
                            if ko == 0:
                                if i == 0:
                                    anchor = m
                                elif not _NODEPS:
                                    add_dep_helper(m.ins, anchor.ins, False, "vt order")
                    gt = tcnk * 4 + tp2 * 2
                    nc.vector.tensor_copy(
                        vaug[:, gt : gt + 2, :, 0:64],
                        slot[:, :].rearrange("p (i x) -> p i x", x=256)[
                            :, :, 0:GCH
                        ].rearrange("p i (h d) -> p i h d", h=HPG),
                    )

            def q_chunk(c):
                proj(xqs[c], W_Q2, 128, qT2, c * 512)
                proj(xqs[c], W_Q1, 64, qT1, c * 512)

            def s_lhsT(h, kt):
                ksl = slice(kt * 128, (kt + 1) * 128)
                if h == 0:
                    return kT2[0:64, ksl]
                if h == 1:
                    return kT2[64:128, ksl]
                return kT1[0:64, ksl]

            def s_rhs(h, s, kt=0):
                qsl = slice(s * 256, (s + 1) * 256)
                if h == 0:
                    return qT2[0:64, qsl]
                if h == 1:
                    return qT2[64:128, qsl]
                return qT1[0:64, qsl]

            def start_norm(kind, s, a_ps):
                # pair: a_ps [65, 512] (h0 cols 0:256, h1 256:512); solo 256
                w = 512 if kind == "pair" else 256
                an = rcpool.tile([65, 512], F32, tag="an")
                nc.vector.tensor_copy(an[:, 0:w], a_ps[0:65, 0:w])
                nc.vector.reciprocal(an[64:65, 0:w], an[64:65, 0:w])
                rcb = rcpool.tile([65, 512], BF16, tag="rcb")
                nc.vector.tensor_copy(rcb[64:65, 0:w], an[64:65, 0:w])
                return (kind, s, an, rcb)

            def finish_norm(kind, s, an, rcb):
                qsl = slice(s * 256, (s + 1) * 256)
                w = 512 if kind == "pair" else 256
                r_ps = aux.tile([128, 512], F32, tag="aux", name="rep")[0:64, :]
                nc.tensor.matmul(
                    r_ps[:, 0:w],
                    ones65[64:65, :],
                    rcb[64:65, 0:w],
                    start=True,
                    stop=True,
                )
                hs = (0, 1) if kind == "pair" else (2,)
                for i, h in enumerate(hs):
                    nc.vector.tensor_tensor(
                        attnT[h][:, qsl],
                        an[0:64, i * 256 : (i + 1) * 256],
                        r_ps[:, i * 256 : (i + 1) * 256],
                        ALU.mult,
                    )

            # pipeline state
            pend_pv = []    # (s, kind, nkt, a_ps, pe_t, ops)
            pend_norm = []  # (due_batch, norm_args)
            batch_no = [0]

            def flush_pv(keep):
                while len(pend_pv) > keep:
                    s, kind, nkt, a_ps, pe_t, ops, pv_first = pend_pv.pop(0)
                    for h, kt, j, c0, pc0 in ops:
                        first = (kt == 0) and not pv_first
                        m = nc.tensor.matmul(
                            a_ps[0:65, pc0 : pc0 + 256],
                            vaug[:, kt, h, 0:65],
                            pe_t[:, j, c0 : c0 + 256],
                            start=first,
                            stop=(kt == nkt - 1),
                            skip_group_check=not first,
                        )
                        if first:
                            pv_first.append(m)
                        elif kt == 0 and not _NODEPS:
                            add_dep_helper(m.ins, pv_first[0].ins, False, "aps order")
                    if any(kt == nkt - 1 for _, kt, _, _, _ in ops):
                        pend_norm.append(
                            (batch_no[0] + 4, start_norm(kind, s, a_ps))
                        )

            def flush_norms(force=False):
                while pend_norm and (force or pend_norm[0][0] <= batch_no[0]):
                    _, args = pend_norm.pop(0)
                    finish_norm(*args)

            def emit_phaseD(ts):
                tsl = slice(ts * 512, (ts + 1) * 512)
                for oc in range(C // 128):
                    ocs = slice(oc * 128, (oc + 1) * 128)
                    po = aux.tile([128, 512], F32, tag="aux", name="po")
                    for h in range(HPG):
                        nc.tensor.matmul(
                            po[:],
                            wo_sb[h][:, ocs],
                            attnT[h][:, tsl],
                            start=(h == 0),
                            stop=(h == HPG - 1),
                        )
                    ob = ob_pool.tile([128, 512], F32, tag="ob")
                    nc.vector.tensor_copy(ob[:], po[:])
                    nc.sync.dma_start(out[ocs, tsl], ob[:])

            unit_list = []
            for s in range(NS):
                unit_list.append(("kv", s))
                if s == 0:
                    unit_list.append(("q", 0))
                if _STOP_AFTER != "AB":
                    unit_list.append((s, "pair"))
                    unit_list.append((s, "solo"))
                if s % 2 == 1 and s < 7:
                    unit_list.append(("q", (s + 1) // 2))

            for s, kind in unit_list:
                if s == "kv":
                    kv_chunk(kind)
                    continue
                if s == "q":
                    q_chunk(kind)
                    continue
                nkt = 4 * s + 4
                flush_norms(force=True)
                # phase D for query chunk ts slots in once the norms for
                # its supertiles (2ts, 2ts+1) are guaranteed flushed: at the
                # SOLO unit of supertile 2ts+2 (the pair unit of 2ts+2 pops
                # all of 2ts+1's pending PV batches through the LAG window,
                # and this unit's force-flush above emitted the norms).
                if (
                    kind == "solo" and s >= 2 and s % 2 == 0
                    and _STOP_AFTER == "full"
                ):
                    emit_phaseD((s - 2) // 2)
                a_ps = apsp.tile([65, 512], F32, tag="attn", name="a_ps")
                pv_first = []
                step = 2 if kind == "pair" else 4
                for kt0 in range(0, nkt, step):
                    kts = list(range(kt0, kt0 + step))
                    bs = sps.tile([128, BK, 512], F32, tag="s")
                    # ops: (head, kt, bank j, score col c0, pv col pc0)
                    if kind == "pair":
                        # bank h holds head h's kt pair col-packed; same-bank
                        # writers share tile_position, row-tiled heads write
                        # different banks
                        ops = []
                        for i, kt in enumerate(kts):
                            ops += [(0, kt, 0, i * 256, 0), (1, kt, 1, i * 256, 256)]
                    else:
                        ops = [
                            (2, kt, idx // 2, (idx % 2) * 256, 0)
                            for idx, kt in enumerate(kts)
                        ]
                    bank_first = {}
                    for h, kt, j, c0, pc0 in ops:
                        tail = kt >= 4 * s
                        first = j not in bank_first
                        m = nc.tensor.matmul(
                            bs[:, j, c0 : c0 + 256],
                            s_lhsT(h, kt), s_rhs(h, s, kt),
                            start=first, stop=not tail,
                            skip_group_check=not first,
                        )
                        if first:
                            bank_first[j] = m
                        elif not _NODEPS:
                            add_dep_helper(m.ins, bank_first[j].ins, False, "bank order")
                        if tail:
                            r = kt - 4 * s
                            nc.tensor.matmul(
                                bs[:, j, c0 : c0 + 256], ident[:], tm_sb[:, r, :],
                                start=False, stop=True,
                                skip_group_check=True,
                            )
                    batch_no[0] += 1
                    flush_pv(LAG)
                    flush_norms()
                    pe_t = pepool.tile([128, BK, 512], BF16, tag="pe")
                    nc.scalar.activation(pe_t[:], bs[:], AF.Exp, scale=0.125)
                    pend_pv.append((s, kind, nkt, a_ps, pe_t, ops, pv_first))
            flush_pv(0)
            flush_norms(force=True)
            if _STOP_AFTER == "full":
                emit_phaseD(3)

    nc.compile()
    return nc


def _get_nc():
    if "nc" not in _CACHE:
        _CACHE["nc"] = build_nc()
    return _CACHE["nc"]


BF = ml_dtypes.bfloat16


def pack_w(w):
    """[C, n] -> [128, KO*n] so each SBUF partition row is contiguous."""
    n = w.shape[1]
    return np.ascontiguousarray(
        w.reshape(KO, 128, n).transpose(1, 0, 2).reshape(128, KO * n)
    ).astype(BF)


def make_in_maps(inputs):
    """Shard full inputs into 8 per-core input maps.

    xT [C, T] is the host-transposed bf16 x, shared by all cores (k/v need
    every key row).  xqT [C, TQ] is the parity-gathered query view: core
    parity qh owns global 128-row q blocks {2j+qh}, laid out ascending.

    tmask [128k, r, 256q] covers the 4 tail kts (r = kt - 4s) of each
    256-query supertile s.  Local q block j (j=0,1) of supertile s is
    global block 4s+2j+qh; tail kt 4s+r is global key block 4s+r, so
    delta = r - 2j - qh: 0 -> diagonal triangle mask, >0 -> fully masked,
    <0 -> keep (zeros).
    """
    x = np.ascontiguousarray(np.asarray(inputs["x"], dtype=np.float32)).reshape(T, C)
    W_qkv = np.asarray(inputs["W_qkv"], dtype=np.float32)
    W_out = np.asarray(inputs["W_out"], dtype=np.float32)

    NEG = np.float32(-1e9)
    diag_add = np.where(
        np.arange(128)[None, :] >= np.arange(128)[:, None], np.float32(0), NEG
    )  # [k, q]: keep q >= k

    xT = np.ascontiguousarray(x.T).astype(BF)  # [C, T]
    xr = x.reshape(NTT, 128, C)
    xqT = {
        qh: np.ascontiguousarray(xr[qh::2].reshape(TQ, C).T).astype(BF)
        for qh in (0, 1)
    }

    tmask = {}
    for qh in (0, 1):
        m = np.zeros((128, 4, 256), np.float32)
        for r in range(4):
            for j in range(2):
                delta = r - 2 * j - qh
                blk = m[:, r, j * 128 : (j + 1) * 128]
                if delta == 0:
                    blk[:] = diag_add
                elif delta > 0:
                    blk[:] = NEG
        tmask[qh] = m.astype(BF)

    in_maps = []
    for c in range(N_CORES):
        g, qh = c // 2, c % 2
        in_maps.append(
            {
                "xT": xT,
                "xqT": xqT[qh],
                "wpack": pack_w(
                    np.concatenate(
                        [
                            W_qkv[:, 1 * C + g * GCH : 1 * C + (g + 1) * GCH],
                            W_qkv[:, 2 * C + g * GCH : 2 * C + (g + 1) * GCH],
                            W_qkv[:, 0 * C + g * GCH : 0 * C + (g + 1) * GCH],
                        ],
                        axis=1,
                    )
                ),
                "wo": np.ascontiguousarray(W_out[g * GCH : (g + 1) * GCH, :]).astype(BF),
                "tmask": tmask[qh],
            }
        )
    return in_maps


def combine_outputs(parts, b_out):
    """Sum head-group partials per parity, reassemble rows, add bias."""
    NQT = TQ // 128
    out = np.zeros((T, C), np.float32)
    orow = out.reshape(NTT, 128, C)
    for qh in (0, 1):
        acc = parts[qh].astype(np.float32).copy()
        for g in range(1, 4):
            acc += parts[2 * g + qh]
        orow[qh::2] = np.ascontiguousarray(acc.T).reshape(NQT, 128, C)
    out += np.asarray(b_out, dtype=np.float32)[None, :]
    return out.reshape(1, T, C)


def _run(inputs, trace=False, tmpdir=None):
    nc = _get_nc()
    in_maps = make_in_maps(inputs)
    res = bass_utils.run_bass_kernel_spmd(
        nc, in_maps, core_ids=list(range(N_CORES)), trace=trace, tmpdir=tmpdir
    )
    parts = [np.asarray(res.results[c]["out"]) for c in range(N_CORES)]
    return combine_outputs(parts, inputs["b_out"]), res


def kernel(**inputs):
    out, _ = _run(inputs)
    return out


# revision 26
# speedup vs baseline: 1.1525x; 1.0102x over previous
"""Causal self-attention (B=1, T=4096, C=768, H=12, D=64) on 8 TRN2 NeuronCores.

Sharding: 4 head-groups x 2 query-parity sets (core c: group g=c//2 owning
heads 3g..3g+2, parity qh=c%2 owning query blocks {2j+qh}).  The host sums
the 4 head-group output partials per parity, adds b_out, and reassembles
the interleaved rows.  All SPMD cores run one identical program; per-core
variation enters only through data.

v2 (vs the fp32r baseline):
  - all PE operands are bf16 (PSUM stays fp32): FWL fast-weight-loads
    engage, matmuls run 1 cycle/row unconditionally, DMA and DVE halve.
  - x is pre-transposed AND pre-cast on the host (xT [C,T] bf16): zero
    on-device transposes.  v is produced directly in [t,d] layout by using
    xT tiles as the matmul stationary.
  - 256-query supertiles (2 parity blocks, kt span 4) cut causal
    overcompute from 22% to 9% on scores, exp and PV.
  - head-0/1 score matmuls are row-tiled (K=64 halves of the PE array via
    base partitions 0/64) so they execute concurrently.
  - two heads (or two kts) share one PSUM bank: first matmul start=True
    (whole-bank pending-zero), second start=False overwrites its
    untouched half (explicit add_dep_helper edge pins the order).  exp
    then covers [128,2,512] = 1024 free elements per ACT instruction.
  - softmax denominators still ride the ones-column in the PV stationary;
    reciprocal via reciprocal_approx_fast (~5x the DVE divide).
"""

import numpy as np
import ml_dtypes
from contextlib import ExitStack

import concourse.bass as bass  # noqa: F401
import concourse.mybir as mybir
import concourse.tile as tile
from concourse import bacc
from concourse import bass_utils
from concourse.masks import make_identity
from concourse.tile_rust import add_dep_helper

T, C, H, D = 4096, 768, 12, 64
N_CORES = 8
HPG = 3                    # heads per group
GCH = HPG * D              # 192 channels per group per tensor
TQ = T // 2                # 2048 query rows per core
NTT = T // 128             # 32 key tiles
KO = C // 128              # 6 contraction subtiles
NS = TQ // 256             # 8 query supertiles per core (256 q each)

F32 = mybir.dt.float32
F32R = mybir.dt.float32r
BF16 = mybir.dt.bfloat16
AF = mybir.ActivationFunctionType
ALU = mybir.AluOpType

_CACHE = {}
_STOP_AFTER = "full"  # "AB" | "C" | "full"
import os
_NODEPS = os.environ.get("BISECT_NODEPS", "0") == "1"
_NOSHARE = os.environ.get("BISECT_NOSHARE", "0") == "1"


def build_nc():
    nc = bacc.Bacc(
        "TRN2", target_bir_lowering=False, debug=False, num_devices=N_CORES
    )

    xT_d = nc.dram_tensor("xT", [C, T], BF16, kind="ExternalInput").ap()
    xqT_d = nc.dram_tensor("xqT", [C, TQ], BF16, kind="ExternalInput").ap()
    # packed qkv weights: per-ko concat [wk2|wk1|wv3|wq2|wq1] = 576 cols
    wp_d = nc.dram_tensor("wpack", [128, KO * 576], BF16, kind="ExternalInput").ap()
    wo_d = nc.dram_tensor("wo", [GCH, C], BF16, kind="ExternalInput").ap()
    tm_d = nc.dram_tensor("tmask", [128, 4, 256], BF16, kind="ExternalInput").ap()
    out = nc.dram_tensor("out", [C, TQ], BF16, kind="ExternalOutput").ap()

    with tile.TileContext(nc) as tc, ExitStack() as ctx:
        wpool = ctx.enter_context(tc.tile_pool(name="weights", bufs=1))
        dpool = ctx.enter_context(tc.tile_pool(name="data", bufs=1))

        # --- weights / constants ---
        # weights arrive host-packed: one DMA, contiguous partition rows
        wp_sb = wpool.tile([128, KO, 576], BF16, name="wp_sb")
        nc.sync.dma_start(wp_sb[:], wp_d.rearrange("p (ko n) -> p ko n", n=576))
        W_K2, W_K1, W_V3, W_Q2, W_Q1 = (
            (0, 128), (128, 192), (192, 384), (384, 512), (512, 576)
        )
        wo_sb = [wpool.tile([64, C], BF16, name=f"wo{h}") for h in range(HPG)]
        tm_sb = wpool.tile([128, 4, 256], BF16, name="tm_sb")

        ident32 = wpool.tile([128, 128], F32, name="ident32")
        make_identity(nc, ident32[:])
        ident = wpool.tile([128, 128], BF16, name="ident")
        nc.vector.tensor_copy(ident[:], ident32[:])
        ones65_32 = wpool.tile([65, 64], F32, name="ones65_32")
        nc.vector.memset(ones65_32[:], 1.0)
        ones65 = wpool.tile([65, 64], BF16, name="ones65")
        nc.vector.tensor_copy(ones65[:], ones65_32[:])

        # --- persistent tensors ---
        qT2 = dpool.tile([128, TQ], BF16, name="qT2")     # q heads 0,1 [d,t]
        qT1 = dpool.tile([64, TQ], BF16, name="qT1")      # q head 2
        kT2 = dpool.tile([128, T], BF16, name="kT2")      # k heads 0,1
        kT1 = dpool.tile([64, T], BF16, name="kT1")       # k head 2
        vaug = dpool.tile([128, NTT, HPG, 65], BF16, name="vaug")  # [t,d]+ones
        attnT = [dpool.tile([64, TQ], BF16, name=f"aT{h}") for h in range(HPG)]
        nc.vector.memset(vaug[:, :, :, 64:65], 1.0)

        # --- phases A-D, emission interleaved per supertile so the PE
        # projects chunk s+1 while ACT exps unit s, and exp work starts as
        # soon as chunk 0 lands instead of after the whole projection pass.
        BK = 2   # kt slots per psum tile (pair units: 1 kt/bank x 2 heads;
                 # solo units: 2 kts/bank col-packed)
        LAG = 2  # batches between scores and PV
        with (
            tc.tile_pool(name="xchunk", bufs=12) as xpool,
            tc.tile_pool(name="pe", bufs=4 + LAG) as pepool,
            tc.tile_pool(name="rc", bufs=4) as rcpool,
            tc.tile_pool(name="s_ps", bufs=2, space="PSUM") as sps,
            tc.tile_pool(name="a_ps", bufs=2, space="PSUM") as apsp,
            tc.tile_pool(name="x_ps", bufs=2, space="PSUM") as aux,
            tc.tile_pool(name="ob", bufs=3) as ob_pool,
        ):
            # DMAs up front, in consumption order
            nc.sync.dma_start(tm_sb[:], tm_d[:])
            xts, xqs = [None] * 8, [None] * 4

            def dma_xt(i):
                xt = xpool.tile([128, KO, 512], BF16, tag="xt", name=f"xt{i}")
                nc.scalar.dma_start(
                    xt[:],
                    xT_d[:, i * 512 : (i + 1) * 512].rearrange(
                        "(ko p) t -> p ko t", p=128
                    ),
                )
                xts[i] = xt

            def dma_xq(c):
                xq = xpool.tile([128, KO, 512], BF16, tag="xt", name=f"xq{c}")
                nc.scalar.dma_start(
                    xq[:],
                    xqT_d[:, c * 512 : (c + 1) * 512].rearrange(
                        "(ko p) t -> p ko t", p=128
                    ),
                )
                xqs[c] = xq

            dma_xq(0), dma_xt(0), dma_xt(1), dma_xq(1)
            for h in range(HPG):
                nc.sync.dma_start(wo_sb[h][:], wo_d[h * 64 : (h + 1) * 64, :])
            dma_xt(2), dma_xt(3), dma_xq(2), dma_xt(4), dma_xt(5)
            dma_xq(3), dma_xt(6), dma_xt(7)

            def proj(xt, wcols, m, dest, off):
                """dest[:, off:off+512] = wp[:, :, wcols].T @ xt over ko."""
                lo, hi = wcols
                slot = aux.tile([128, 512], F32, tag="aux", name="projps")
                ps = slot[0:m, :]
                for ko in range(KO):
                    nc.tensor.matmul(
                        ps,
                        wp_sb[:, ko, lo:hi],
                        xt[:, ko, :],
                        start=(ko == 0),
                        stop=(ko == KO - 1),
                    )
                nc.vector.tensor_copy(dest[:, off : off + 512], ps)

            def kv_chunk(tcnk):
                xt = xts[tcnk]
                t0 = tcnk * 512
                proj(xt, W_K2, 128, kT2, t0)
                proj(xt, W_K1, 64, kT1, t0)
                # v in [t, d] layout: xT tile stationary, Wv moving;
                # two t-tiles col-packed per psum bank (same tile_position)
                for tp2 in range(2):
                    slot = aux.tile([128, 512], F32, tag="aux", name="vtps")
                    anchor = None
                    for i in range(2):
                        tt = tp2 * 2 + i
                        vt = slot[:, i * 256 : i * 256 + GCH]
                        for ko in range(KO):
                            m = nc.tensor.matmul(
                                vt,
                                xt[:, ko, tt * 128 : (tt + 1) * 128],
                                wp_sb[:, ko, W_V3[0] : W_V3[1]],
                                start=(ko == 0 and i == 0),
                                stop=(ko == KO - 1),
                                skip_group_check=(i == 1),
                            )
                            if ko == 0:
                                if i == 0:
                                    anchor = m
                                elif not _NODEPS:
                                    add_dep_helper(m.ins, anchor.ins, False, "vt order")
                    gt = tcnk * 4 + tp2 * 2
                    nc.vector.tensor_copy(
                        vaug[:, gt : gt + 2, :, 0:64],
                        slot[:, :].rearrange("p (i x) -> p i x", x=256)[
                            :, :, 0:GCH
                        ].rearrange("p i (h d) -> p i h d", h=HPG),
                    )

            def q_chunk(c):
                proj(xqs[c], W_Q2, 128, qT2, c * 512)
                proj(xqs[c], W_Q1, 64, qT1, c * 512)

            def s_lhsT(h, kt):
                ksl = slice(kt * 128, (kt + 1) * 128)
                if h == 0:
                    return kT2[0:64, ksl]
                if h == 1:
                    return kT2[64:128, ksl]
                return kT1[0:64, ksl]

            def s_rhs(h, s, kt=0):
                qsl = slice(s * 256, (s + 1) * 256)
                if h == 0:
                    return qT2[0:64, qsl]
                if h == 1:
                    return qT2[64:128, qsl]
                return qT1[0:64, qsl]

            def start_norm(kind, s, a_ps):
                # pair: a_ps [65, 512] (h0 cols 0:256, h1 256:512); solo 256
                w = 512 if kind == "pair" else 256
                an = rcpool.tile([65, 512], F32, tag="an")
                nc.vector.tensor_copy(an[:, 0:w], a_ps[0:65, 0:w])
                nc.vector.reciprocal(an[64:65, 0:w], an[64:65, 0:w])
                rcb = rcpool.tile([65, 512], BF16, tag="rcb")
                nc.vector.tensor_copy(rcb[64:65, 0:w], an[64:65, 0:w])
                return (kind, s, an, rcb)

            def finish_norm(kind, s, an, rcb):
                qsl = slice(s * 256, (s + 1) * 256)
                w = 512 if kind == "pair" else 256
                r_ps = aux.tile([128, 512], F32, tag="aux", name="rep")[0:64, :]
                nc.tensor.matmul(
                    r_ps[:, 0:w],
                    ones65[64:65, :],
                    rcb[64:65, 0:w],
                    start=True,
                    stop=True,
                )
                hs = (0, 1) if kind == "pair" else (2,)
                for i, h in enumerate(hs):
                    nc.vector.tensor_tensor(
                        attnT[h][:, qsl],
                        an[0:64, i * 256 : (i + 1) * 256],
                        r_ps[:, i * 256 : (i + 1) * 256],
                        ALU.mult,
                    )

            # pipeline state
            pend_pv = []    # (s, kind, nkt, a_ps, pe_t, ops)
            pend_norm = []  # (due_batch, norm_args)
            batch_no = [0]

            def flush_pv(keep):
                while len(pend_pv) > keep:
                    s, kind, nkt, a_ps, pe_t, ops, pv_first = pend_pv.pop(0)
                    for h, kt, j, c0, pc0 in ops:
                        first = (kt == 0) and not pv_first
                        m = nc.tensor.matmul(
                            a_ps[0:65, pc0 : pc0 + 256],
                            vaug[:, kt, h, 0:65],
                            pe_t[:, j, c0 : c0 + 256],
                            start=first,
                            stop=(kt == nkt - 1),
                            skip_group_check=not first,
                        )
                        if first:
                            pv_first.append(m)
                        elif kt == 0 and not _NODEPS:
                            add_dep_helper(m.ins, pv_first[0].ins, False, "aps order")
                    if any(kt == nkt - 1 for _, kt, _, _, _ in ops):
                        pend_norm.append(
                            (batch_no[0] + 4, start_norm(kind, s, a_ps))
                        )

            def flush_norms(force=False):
                while pend_norm and (force or pend_norm[0][0] <= batch_no[0]):
                    _, args = pend_norm.pop(0)
                    finish_norm(*args)

            def emit_phaseD(ts):
                tsl = slice(ts * 512, (ts + 1) * 512)
                for oc in range(C // 128):
                    ocs = slice(oc * 128, (oc + 1) * 128)
                    po = aux.tile([128, 512], F32, tag="aux", name="po")
                    for h in range(HPG):
                        nc.tensor.matmul(
                            po[:],
                            wo_sb[h][:, ocs],
                            attnT[h][:, tsl],
                            start=(h == 0),
                            stop=(h == HPG - 1),
                        )
                    ob = ob_pool.tile([128, 512], BF16, tag="ob")
                    nc.vector.tensor_copy(ob[:], po[:])
                    nc.sync.dma_start(out[ocs, tsl], ob[:])

            unit_list = []
            for s in range(NS):
                unit_list.append(("kv", s))
                if s == 0:
                    unit_list.append(("q", 0))
                if _STOP_AFTER != "AB":
                    unit_list.append((s, "pair"))
                    unit_list.append((s, "solo"))
                if s % 2 == 1 and s < 7:
                    unit_list.append(("q", (s + 1) // 2))

            for s, kind in unit_list:
                if s == "kv":
                    kv_chunk(kind)
                    continue
                if s == "q":
                    q_chunk(kind)
                    continue
                nkt = 4 * s + 4
                flush_norms(force=True)
                # phase D for query chunk ts slots in once the norms for
                # its supertiles (2ts, 2ts+1) are guaranteed flushed: at the
                # SOLO unit of supertile 2ts+2 (the pair unit of 2ts+2 pops
                # all of 2ts+1's pending PV batches through the LAG window,
                # and this unit's force-flush above emitted the norms).
                if (
                    kind == "solo" and s >= 2 and s % 2 == 0
                    and _STOP_AFTER == "full"
                ):
                    emit_phaseD((s - 2) // 2)
                a_ps = apsp.tile([65, 512], F32, tag="attn", name="a_ps")
                pv_first = []
                step = 2 if kind == "pair" else 4
                for kt0 in range(0, nkt, step):
                    kts = list(range(kt0, kt0 + step))
                    bs = sps.tile([128, BK, 512], F32, tag="s")
                    # ops: (head, kt, bank j, score col c0, pv col pc0)
                    if kind == "pair":
                        # bank h holds head h's kt pair col-packed; same-bank
                        # writers share tile_position, row-tiled heads write
                        # different banks
                        ops = []
                        for i, kt in enumerate(kts):
                            ops += [(0, kt, 0, i * 256, 0), (1, kt, 1, i * 256, 256)]
                    else:
                        ops = [
                            (2, kt, idx // 2, (idx % 2) * 256, 0)
                            for idx, kt in enumerate(kts)
                        ]
                    bank_first = {}
                    for h, kt, j, c0, pc0 in ops:
                        tail = kt >= 4 * s
                        first = j not in bank_first
                        m = nc.tensor.matmul(
                            bs[:, j, c0 : c0 + 256],
                            s_lhsT(h, kt), s_rhs(h, s, kt),
                            start=first, stop=not tail,
                            skip_group_check=not first,
                        )
                        if first:
                            bank_first[j] = m
                        elif not _NODEPS:
                            add_dep_helper(m.ins, bank_first[j].ins, False, "bank order")
                        if tail:
                            r = kt - 4 * s
                            nc.tensor.matmul(
                                bs[:, j, c0 : c0 + 256], ident[:], tm_sb[:, r, :],
                                start=False, stop=True,
                                skip_group_check=True,
                            )
                    batch_no[0] += 1
                    flush_pv(LAG)
                    flush_norms()
                    pe_t = pepool.tile([128, BK, 512], BF16, tag="pe")
                    nc.scalar.activation(pe_t[:], bs[:], AF.Exp, scale=0.125)
                    pend_pv.append((s, kind, nkt, a_ps, pe_t, ops, pv_first))
            flush_pv(0)
            flush_norms(force=True)
            if _STOP_AFTER == "full":
                emit_phaseD(3)

    nc.compile()
    return nc


def _get_nc():
    if "nc" not in _CACHE:
        _CACHE["nc"] = build_nc()
    return _CACHE["nc"]


BF = ml_dtypes.bfloat16


def pack_w(w):
    """[C, n] -> [128, KO*n] so each SBUF partition row is contiguous."""
    n = w.shape[1]
    return np.ascontiguousarray(
        w.reshape(KO, 128, n).transpose(1, 0, 2).reshape(128, KO * n)
    ).astype(BF)


def make_in_maps(inputs):
    """Shard full inputs into 8 per-core input maps.

    xT [C, T] is the host-transposed bf16 x, shared by all cores (k/v need
    every key row).  xqT [C, TQ] is the parity-gathered query view: core
    parity qh owns global 128-row q blocks {2j+qh}, laid out ascending.

    tmask [128k, r, 256q] covers the 4 tail kts (r = kt - 4s) of each
    256-query supertile s.  Local q block j (j=0,1) of supertile s is
    global block 4s+2j+qh; tail kt 4s+r is global key block 4s+r, so
    delta = r - 2j - qh: 0 -> diagonal triangle mask, >0 -> fully masked,
    <0 -> keep (zeros).
    """
    x = np.ascontiguousarray(np.asarray(inputs["x"], dtype=np.float32)).reshape(T, C)
    W_qkv = np.asarray(inputs["W_qkv"], dtype=np.float32)
    W_out = np.asarray(inputs["W_out"], dtype=np.float32)

    NEG = np.float32(-1e9)
    diag_add = np.where(
        np.arange(128)[None, :] >= np.arange(128)[:, None], np.float32(0), NEG
    )  # [k, q]: keep q >= k

    xT = np.ascontiguousarray(x.T).astype(BF)  # [C, T]
    xr = x.reshape(NTT, 128, C)
    xqT = {
        qh: np.ascontiguousarray(xr[qh::2].reshape(TQ, C).T).astype(BF)
        for qh in (0, 1)
    }

    tmask = {}
    for qh in (0, 1):
        m = np.zeros((128, 4, 256), np.float32)
        for r in range(4):
            for j in range(2):
                delta = r - 2 * j - qh
                blk = m[:, r, j * 128 : (j + 1) * 128]
                if delta == 0:
                    blk[:] = diag_add
                elif delta > 0:
                    blk[:] = NEG
        tmask[qh] = m.astype(BF)

    in_maps = []
    for c in range(N_CORES):
        g, qh = c // 2, c % 2
        in_maps.append(
            {
                "xT": xT,
                "xqT": xqT[qh],
                "wpack": pack_w(
                    np.concatenate(
                        [
                            W_qkv[:, 1 * C + g * GCH : 1 * C + (g + 1) * GCH],
                            W_qkv[:, 2 * C + g * GCH : 2 * C + (g + 1) * GCH],
                            W_qkv[:, 0 * C + g * GCH : 0 * C + (g + 1) * GCH],
                        ],
                        axis=1,
                    )
                ),
                "wo": np.ascontiguousarray(W_out[g * GCH : (g + 1) * GCH, :]).astype(BF),
                "tmask": tmask[qh],
            }
        )
    return in_maps


def combine_outputs(parts, b_out):
    """Sum head-group partials per parity, reassemble rows, add bias."""
    NQT = TQ // 128
    out = np.zeros((T, C), np.float32)
    orow = out.reshape(NTT, 128, C)
    for qh in (0, 1):
        acc = parts[qh].astype(np.float32).copy()
        for g in range(1, 4):
            acc += parts[2 * g + qh]
        orow[qh::2] = np.ascontiguousarray(acc.T).reshape(NQT, 128, C)
    out += np.asarray(b_out, dtype=np.float32)[None, :]
    return out.reshape(1, T, C)


def _run(inputs, trace=False, tmpdir=None):
    nc = _get_nc()
    in_maps = make_in_maps(inputs)
    res = bass_utils.run_bass_kernel_spmd(
        nc, in_maps, core_ids=list(range(N_CORES)), trace=trace, tmpdir=tmpdir
    )
    parts = [np.asarray(res.results[c]["out"]) for c in range(N_CORES)]
    return combine_outputs(parts, inputs["b_out"]), res


def kernel(**inputs):
    out, _ = _run(inputs)
    return out
